# revision 16
# baseline (speedup 1.0000x reference)
"""ARCH-III: per-edge bilinear scoring, 2-D sharded, tgt-side selection-matmul.

Core c = (sc, tc): sc = c//2 owns src rows [sc*25000, +25000) (gather side,
int16-safe y_tab of 25088 rows); tc = c%2 owns tgt rows [tc*50000, +50000)
(stream side, 64-row windows -> selT/xtw bytes halved vs 128-row windows).

Per core (edges with src in chunk sc AND tgt in chunk tc, tgt-sorted):
  - Phase 1: y = x_source[chunk] @ W on PE -> y_tab (bf16, DRAM).
  - Phase 2, per 64-row tgt window (static schedule of TPW=3 tiles/window):
      PE:  expanded_xt[e, f] = selT^T @ xt_window    (selT: host-built fp8
           one-hot [64 window_row, edge_slot]; xt_window streamed bf16)
      GPSIMD: dma_gather ys rows (bf16) per 4-window block (pads gather
           row 0; runtime-count trimming / single_packet hang real HW).
      ACT: psum fp32 -> bf16 copy (3 tiles per window-block)
      DVE: prod = expanded_xt * ys ; half-add (2x) ; reduce_64 per block
  - Pad slots (~17%) are discarded by the host on un-permute.
  - Windows whose edge count exceeds TPW*128 spill to host numpy (rare).

Bottleneck (measured): DMA-engine aggregate throughput — gather descriptors
(2 per idx at ~18 ns) + stream bytes. selT/xtw byte cuts move the wall 1:1.
"""

import contextlib
import os
import sys

for _p in ("/opt/trn_rl_repo",):
    if os.path.isdir(_p) and _p not in sys.path:
        sys.path.insert(0, _p)

import ml_dtypes
import numpy as np

import concourse.bacc as bacc
import concourse.bass as bass
import concourse.mybir as mybir
from concourse.bass_utils import run_bass_kernel_spmd
from concourse.library_config import mlp

F32 = mybir.dt.float32
BF16 = mybir.dt.bfloat16
FP8 = mybir.dt.float8e4
I16 = mybir.dt.int16
NP_BF16 = ml_dtypes.bfloat16
NP_FP8 = ml_dtypes.float8_e4m3

N = 100000
E = 2000000
D = 128
NCORES = 8
NSC = 4                     # src chunks
NTC = 2                     # tgt chunks
CHS = N // NSC              # 25000 src rows per chunk
CHP = 25088                 # padded (196 tiles of 128)
TGC = N // NTC              # 50000 tgt rows per chunk
WROWS = 64                  # tgt window rows
WPB = 4                     # windows per gather block
TPW = 3                     # tiles (x128 slots) per window
GW = 8                      # windows per stream group
GT = GW * TPW               # tiles per group (24)
NWIN = (TGC + WROWS - 1) // WROWS        # 782 real windows
NGRP = (NWIN + GW - 1) // GW             # 98 -> pad windows to 98*8=784
NWINP = NGRP * GW                        # 784
NTILES = NWINP * TPW                     # 2352
CAP = NTILES * 128                       # 301056 slots
NI = WPB * TPW * 128        # idxs per gather call (1536)
BTI = WPB * TPW             # tiles per gather block / vector group (12)
NBG = 8                     # ys gather buffer pool (blocks; multiple of NQ
                            # so each gys sem stays on one SWDGE queue)
NXP = 8                     # expanded-xt group-buffer pool (blocks)
NSB = 3                     # sel/xtw stream buffers (triple buffer)
NPX = 16                    # psum expansion slots (4 banks; 4-slot stride
                            # keeps each window-block's 3 tiles in one bank)
NQ = 4
assert NWINP % WPB == 0 and NBG % NQ == 0

_NC_CACHE = None
_LAST_EXEC_NS = None
_TRACE = False

# NOTE: runtime num_idxs_reg (value_load per gather) hangs real HW — pads
# use index 0 and the full NI literal instead. single_packet=True also
# crashes NRT (measured).
SEL_FP8 = os.environ.get("V2_SEL_FP8", "1") == "1"   # fp8 selT (else bf16)
SEL_DT = FP8 if SEL_FP8 else BF16
NP_SEL = NP_FP8 if SEL_FP8 else NP_BF16


def _build_nc_v3(chp=CHP, nwinp=NWINP, ngrp=NGRP, num_devices=NCORES):
    nwin_t = nwinp * TPW               # tiles
    cap = nwin_t * 128
    scols = cap // 16
    n_xtiles = chp // 128
    gt1 = 10 if chp % 1280 == 0 else 4   # phase-1 tiles per group
    grows = gt1 * 128
    ngrp8 = n_xtiles // gt1            # phase-1 groups

    nc = bacc.Bacc("TRN2", target_bir_lowering=False, debug=False,
                   num_devices=num_devices, num_swdge_queues=NQ)
    xsT = nc.dram_tensor("xsT", [D, chp], BF16, kind="ExternalInput")
    W = nc.dram_tensor("W", [D, D], BF16, kind="ExternalInput")
    b_col = nc.dram_tensor("b_col", [D, 1], F32, kind="ExternalInput")
    xtw = nc.dram_tensor("xtw", [WROWS, nwinp, D], BF16, kind="ExternalInput")
    selT = nc.dram_tensor("selT", [WROWS, nwin_t, 128], SEL_DT,
                          kind="ExternalInput")
    src16 = nc.dram_tensor("src16", [128, scols], I16, kind="ExternalInput")
    out = nc.dram_tensor("out", [128, nwin_t], F32, kind="ExternalOutput")
    y_tab = nc.dram_tensor("y_tab", [chp, D], BF16, kind="ExternalOutput")

    with contextlib.ExitStack() as ctx:
        en = ctx.enter_context
        W_sb = en(nc.sbuf_tensor("W_sb", [D, D], BF16))
        b_sb = en(nc.sbuf_tensor("b_sb", [D, 1], F32))
        src_sb = en(nc.sbuf_tensor("src_sb", [128, scols], I16))
        logits = en(nc.sbuf_tensor("logits", [128, nwin_t], F32))
        xs_t = [en(nc.sbuf_tensor(f"xs_t{i}", [D, grows], BF16))
                for i in range(2)]
        y_sb = [en(nc.sbuf_tensor(f"y_sb{i}", [128, gt1, D], BF16))
                for i in range(2)]
        sel_sb = [en(nc.sbuf_tensor(f"sel_sb{i}", [WROWS, GT, 128], SEL_DT))
                  for i in range(NSB)]
        xtw_sb = [en(nc.sbuf_tensor(f"xtw_sb{i}", [WROWS, GW, D], BF16))
                  for i in range(NSB)]
        ys_all = en(nc.sbuf_tensor("ys_all", [128, NBG * BTI, D], BF16))
        xe_sb = [en(nc.sbuf_tensor(f"xe_sb{i}", [128, BTI, D], BF16))
                 for i in range(NXP)]
        prod = [en(nc.sbuf_tensor(f"prod{i}", [128, BTI, D], BF16))
                for i in range(2)]
        s64 = [en(nc.sbuf_tensor(f"s64_{i}", [128, BTI, 64], BF16))
               for i in range(2)]
        y_ps = [en(nc.psum_tensor(f"y_ps{i}", [128, D], F32)) for i in range(4)]
        px = en(nc.psum_tensor("px", [128, NPX, D], F32))

        ld = en(nc.semaphore("ld"))
        xld = [en(nc.semaphore(f"xld{i}")) for i in range(2)]
        mm = en(nc.semaphore("mm"))
        cp = en(nc.semaphore("cp"))
        yst = [en(nc.semaphore(f"yst{i}")) for i in range(2)]
        gsel = [en(nc.semaphore(f"gsel{i}")) for i in range(NSB)]
        gxtw = [en(nc.semaphore(f"gxtw{i}")) for i in range(NSB)]
        psel = [en(nc.semaphore(f"psel{i}")) for i in range(2)]
        pxtw = [en(nc.semaphore(f"pxtw{i}")) for i in range(2)]
        gys = [en(nc.semaphore(f"gys{i}")) for i in range(NBG)]
        mm2 = en(nc.semaphore("mm2"))
        cp2 = en(nc.semaphore("cp2"))
        red = en(nc.semaphore("red"))
        bias = en(nc.semaphore("bias"))
        od = en(nc.semaphore("od"))

        with nc.Block() as block:

            @block.sync
            def _(sync):
                sync.dma_start(out=W_sb[:], in_=W[:]).then_inc(ld, 16)
                sync.dma_start(out=b_sb[:], in_=b_col[:]).then_inc(ld, 16)
                # phase 1: xsT loads / y_tab stores
                for j in range(min(2, ngrp8)):
                    sync.dma_start(out=xs_t[j][:],
                                   in_=xsT[:, j * grows:(j + 1) * grows]
                                   ).then_inc(xld[j], 16)
                for j in range(ngrp8):
                    if j + 2 < ngrp8:
                        sync.wait_ge(mm, gt1 * (j + 1))
                        sync.dma_start(
                            out=xs_t[j % 2][:],
                            in_=xsT[:, (j + 2) * grows:(j + 3) * grows],
                        ).then_inc(xld[j % 2], 16)
                    sync.wait_ge(cp, gt1 * (j + 1))
                    sync.dma_start(
                        out=y_tab[j * grows:(j + 1) * grows, :].rearrange(
                            "(g p) d -> p g d", p=128),
                        in_=y_sb[j % 2][:]).then_inc(yst[j % 2], 16)
                # phase-2 stream: groups >= 2 (prologue pair is on scalar)
                for g in range(2, ngrp):
                    if g >= 3:
                        sync.wait_ge(mm2, GT * (g - 2))
                    sync.dma_start(out=sel_sb[g % NSB][:],
                                   in_=selT[:, g * GT:(g + 1) * GT, :]
                                   ).then_inc(gsel[g % NSB], 16)
                    sync.dma_start(out=xtw_sb[g % NSB][:],
                                   in_=xtw[:, g * GW:(g + 1) * GW, :]
                                   ).then_inc(gxtw[g % NSB], 16)
                sync.wait_ge(bias, 1)
                sync.dma_start(out=out[:], in_=logits[:]).then_inc(od, 16)
                sync.wait_ge(od, 16)

            @block.tensor
            def _(tensor):
                tensor.wait_ge(ld, 48)
                # phase 1: y = xs @ W
                for i in range(n_xtiles):
                    j = i // gt1
                    tensor.wait_ge(xld[j % 2], 16 * (j // 2 + 1))
                    if i >= 4:
                        tensor.wait_ge(cp, i - 3)
                    tensor.matmul(
                        out=y_ps[i % 4][:],
                        lhsT=xs_t[j % 2][:, (i % gt1) * 128:
                                         (i % gt1 + 1) * 128],
                        rhs=W_sb[:], start=True,
                        stop=True).then_inc(mm, 1)
                # phase 2: expansion matmuls, tile-by-tile
                for tau in range(nwin_t):
                    g, tt = tau // GT, tau % GT
                    wg = tt // TPW
                    if tau % GT == 0:
                        if g < 2:
                            tensor.wait_ge(psel[g], 16)
                            tensor.wait_ge(pxtw[g], 16)
                        else:
                            tensor.wait_ge(gsel[g % NSB],
                                           16 * ((g - 2) // NSB + 1))
                            tensor.wait_ge(gxtw[g % NSB],
                                           16 * ((g - 2) // NSB + 1))
                    wb, ti = tau // TPW, tau % TPW
                    if wb >= 4 and ti == 0:
                        tensor.wait_ge(cp2, wb - 3)
                    tensor.matmul(
                        out=px[:, 4 * (wb % 4) + ti, :],
                        lhsT=sel_sb[g % NSB][:, tt, :],
                        rhs=xtw_sb[g % NSB][:, wg, :],
                        start=True, stop=True).then_inc(mm2, 1)

            @block.scalar
            def _(scalar):
                # prologue loads (parallel to sync's phase-1 stream)
                scalar.dma_start(out=src_sb[:], in_=src16[:]).then_inc(ld, 16)
                for g in range(min(2, ngrp)):
                    scalar.dma_start(out=sel_sb[g][:],
                                     in_=selT[:, g * GT:(g + 1) * GT, :]
                                     ).then_inc(psel[g], 16)
                    scalar.dma_start(out=xtw_sb[g][:],
                                     in_=xtw[:, g * GW:(g + 1) * GW, :]
                                     ).then_inc(pxtw[g], 16)
                # per window-block psum->bf16 copy; WPB windows share an xe
                # group buffer
                for wb in range(nwinp):
                    wp = wb // WPB
                    scalar.wait_ge(mm2, TPW * wb + TPW)
                    if wp >= NXP and wb % WPB == 0:
                        scalar.wait_ge(red, wp - (NXP - 1))
                    p0 = 4 * (wb % 4)
                    scalar.activation(
                        out=xe_sb[wp % NXP][:, (wb % WPB) * TPW:
                                            (wb % WPB) * TPW + TPW, :],
                        in_=px[:, p0:p0 + TPW, :],
                        func=mybir.ActivationFunctionType.Copy,
                    ).then_inc(cp2, 1)

            @block.vector
            def _(vector):
                # phase 1: psum -> sbuf copies (bf16)
                for i in range(n_xtiles):
                    j = i // gt1
                    vector.wait_ge(mm, i + 1)
                    if j >= 2 and i % gt1 == 0:
                        vector.wait_ge(yst[j % 2], 16 * (j // 2))
                    vector.tensor_copy(out=y_sb[j % 2][:, i % gt1, :],
                                       in_=y_ps[i % 4][:]).then_inc(cp, 1)
                # phase 2: multiply + half-add + reduce per gather BLOCK
                ngrp2 = nwinp // WPB
                for p in range(ngrp2):
                    vector.wait_ge(cp2, WPB * p + WPB)
                    vector.wait_ge(gys[p % NBG], 16 * (p // NBG + 1))
                    vector.tensor_tensor(
                        out=prod[p % 2][:], in0=xe_sb[p % NXP][:],
                        in1=ys_all[:, (p % NBG) * BTI:
                                   (p % NBG) * BTI + BTI, :],
                        op=mybir.AluOpType.mult)
                    vector.tensor_tensor(
                        out=s64[p % 2][:], in0=prod[p % 2][:, :, 0:64],
                        in1=prod[p % 2][:, :, 64:128],
                        op=mybir.AluOpType.add)
                    vector.tensor_reduce(
                        out=logits[:, BTI * p:BTI * (p + 1)],
                        in_=s64[p % 2][:], axis=mybir.AxisListType.X,
                        op=mybir.AluOpType.add).then_inc(red, 1)
                vector.wait_ge(red, ngrp2)
                vector.tensor_scalar_add(out=logits[:], in0=logits[:],
                                         scalar1=b_sb[:, :1]).then_inc(bias, 1)

            @block.gpsimd
            def _(gpsimd):
                gpsimd.load_library(mlp)
                gpsimd.wait_ge(ld, 48)
                gpsimd.wait_ge(yst[0], 16 * ((ngrp8 + 1) // 2))
                gpsimd.wait_ge(yst[1], 16 * (ngrp8 // 2))
                nblk = nwinp // WPB
                for b in range(nblk):
                    if b >= NBG:
                        gpsimd.wait_ge(red, b - NBG + 1)
                    c0 = b * (NI // 16)
                    gpsimd.dma_gather(
                        ys_all[:, (b % NBG) * BTI:(b % NBG) * BTI + BTI, :],
                        y_tab[:], src_sb[:, c0:c0 + NI // 16],
                        NI, NI, D, single_packet=False,
                        queue_num=b % NQ,
                    ).then_inc(gys[b % NBG], 16)

    nc.compile()
    return nc


def _get_nc():
    global _NC_CACHE
    if _NC_CACHE is None:
        _NC_CACHE = _build_nc_v3()
    return _NC_CACHE


def _idx16_wrap(idx, cap, ni=NI):
    """[cap] int array -> [128, cap//16] int16 in dma_gather layout."""
    nblk = cap // ni
    a = idx.astype(np.int16).reshape(nblk, ni // 16, 16)
    c = a.transpose(0, 2, 1)              # [nblk, 16, ni//16]
    c = np.concatenate(list(c), axis=1)   # [16, nblk*ni//16]
    return np.ascontiguousarray(np.tile(c, (8, 1)))


def host_prep(x_source, x_target, src, tgt, W, bval,
              nwin=NWIN, nwinp=NWINP, cap=CAP):
    """Build per-core input maps. Returns (in_maps, slot_eids, spill_eids)."""
    nwin_t = nwinp * TPW
    b_colv = np.full((D, 1), bval, dtype=np.float32)

    # xtw per tgt chunk: [WROWS, nwinp, D]
    xtw_tc = []
    for tc in range(NTC):
        rows = np.zeros((nwinp * WROWS, D), dtype=np.float32)
        rows[:TGC] = x_target[tc * TGC:(tc + 1) * TGC]
        xtw_tc.append(np.ascontiguousarray(
            rows.reshape(nwinp, WROWS, D).transpose(1, 0, 2)).astype(NP_BF16))
    xsT_sc = []
    for sc in range(NSC):
        m = np.zeros((D, CHP), dtype=NP_BF16)
        m[:, :CHS] = x_source[sc * CHS:(sc + 1) * CHS].T.astype(NP_BF16)
        xsT_sc.append(np.ascontiguousarray(m))

    core_of = (src // CHS) * NTC + (tgt // TGC)
    order = np.lexsort((tgt, core_of))
    bounds = np.searchsorted(core_of[order], np.arange(NCORES + 1))

    in_maps, slot_eids, spill = [], [], []
    for c in range(NCORES):
        sc, tc = divmod(c, NTC)
        eids = order[bounds[c]:bounds[c + 1]]
        s_loc = src[eids] - sc * CHS
        t_loc = tgt[eids] - tc * TGC
        wb_of = t_loc // WROWS
        wbounds = np.searchsorted(wb_of, np.arange(nwin + 1))
        slot_eid = np.full(cap, -1, dtype=np.int64)
        srcI = np.full(cap, -1, dtype=np.int64)       # -1 = pad (trimmed)
        trowI = np.zeros(cap, dtype=np.int64)
        cap_w = TPW * 128
        for j in range(nwin):
            lo, hi = wbounds[j], wbounds[j + 1]
            take = min(hi - lo, cap_w)
            if hi - lo > cap_w:
                spill.extend(eids[lo + cap_w:hi])
            sl = slice(j * cap_w, j * cap_w + take)
            slot_eid[sl] = eids[lo:lo + take]
            srcI[sl] = s_loc[lo:lo + take]
            trowI[sl] = t_loc[lo:lo + take] - j * WROWS
        valid = slot_eid >= 0
        srcI[~valid] = 0
        # selT: [WROWS, NTILES, 128], one-hot per valid slot
        selT = np.zeros((WROWS, nwin_t * 128), dtype=NP_SEL)
        vs = np.nonzero(valid)[0]
        selT[trowI[vs], vs] = 1.0
        selT = selT.reshape(WROWS, nwin_t, 128)

        in_maps.append({
            "xsT": xsT_sc[sc],
            "W": W.astype(NP_BF16),
            "b_col": b_colv,
            "xtw": xtw_tc[tc],
            "selT": selT,
            "src16": _idx16_wrap(srcI, cap),
        })
        slot_eids.append(slot_eid)
    return in_maps, slot_eids, np.array(spill, dtype=np.int64)


def kernel(x_source, x_target, edge_label_index, W, b):
    global _LAST_EXEC_NS
    x_source = np.asarray(x_source, dtype=np.float32)
    x_target = np.asarray(x_target, dtype=np.float32)
    eli = np.asarray(edge_label_index)
    W = np.asarray(W, dtype=np.float32)
    bval = float(np.asarray(b))

    src = eli[0].astype(np.int64)
    tgt = eli[1].astype(np.int64)
    n_edges = src.shape[0]

    in_maps, slot_eids, spill = host_prep(x_source, x_target, src, tgt, W,
                                          bval)
    nc = _get_nc()
    res = run_bass_kernel_spmd(nc, in_maps, core_ids=list(range(NCORES)),
                               trace=_TRACE)
    _LAST_EXEC_NS = res.exec_time_ns

    result = np.empty(n_edges, dtype=np.float32)
    for c in range(NCORES):
        flat = res.results[c]["out"].T.reshape(-1)   # slot = tau*128 + p
        eid = slot_eids[c]
        v = eid >= 0
        result[eid[v]] = flat[v]
    if spill.size:
        xs = x_source[src[spill]]
        xt = x_target[tgt[spill]]
        result[spill] = np.einsum('ed,df,ef->e', xs, W, xt,
                                  optimize=True) + bval
    return result
